# revision 1
# baseline (speedup 1.0000x reference)
"""Cross-attention kernel for Trainium2 (8 NeuronCores, data-parallel over batch).

Problem (hardcoded): B=8, Sq=4096, Sk=77, E=1024, C=768, H=16 heads, D=64.

    q = x @ wq + bq; k = y @ wk + bk; v = y @ wv + bv
    out = softmax(q k^T / sqrt(D)) v @ wo + bo

Sharding: batch element b -> core b. No collectives.

All matmul operands are bf16 (same 1 cycle/row PE rate as fp32r at N=512,
half the DMA traffic and SBUF footprint); PSUM accumulation is fp32.
Measured end-to-end rel-to-scale error ~7e-3 (gate 2e-2).

Per-core pipeline (all matmuls contract over the SBUF partition dim):
  - Activations feature-major: xT[E, Sq] prepared host-side, so the QT
    matmul chain produces qT[E, Sq] directly; per-head slices feed scores
    without on-chip transposes. 1/sqrt(D) folded into wq/bq host-side.
  - Phase 0: kT[E, Sk] via lhsT=wk tiles (et-major, 2 rotating PSUM banks);
    V[Sk, E] row-major via lhsT=yT tiles.
  - scores^T[Sk, q] per head pair: two K=64 matmuls on PE row groups 0-1 /
    2-3 (auto tile_position from base partitions 0/64) run concurrently.
  - exp on ScalarE (no max-subtraction: scores are O(6), fp32 PSUM in,
    bf16 out).
  - attn@V per head pair: two K=77 M=64 matmuls col-tiled into ONE PSUM
    bank (outputs at partitions 0:64 / 64:128) — concurrent via col groups.
    Denominators likewise: lhsT=ones[77,64] pairs into a second bank, so
    rows 0:64 / 64:128 hold den_A / den_B replicated — already in the
    broadcast layout the normalization needs. No one-hot/select matmuls.
  - Normalization fused into PSUM eviction: rbr = reciprocal_approx_fast
    (den bank) on DVE, then oT[:, et, :] = pav * rbr in one DVE
    tensor_tensor (PSUM x SBUF -> bf16). No separate norm phase.
  - out[q, E] = matmul(lhsT=oT tiles, rhs=wo tiles), bo added during
    eviction (DVE), fp32 out.

Schedule (emission order = per-engine execution order):
  QT(0) -> phase0(kT,V) -> for c: [scores(c,et); attnV/den(c,et-1);
  filler(et)] where filler = QT(1) groups at c=0 and final(c-1) groups for
  c>=1; QT(c+1) after chunk c's attention. Fillers keep the PE busy during
  the scores->exp->attnV ScalarE round-trips.
"""

import os
from contextlib import ExitStack

import numpy as np
import ml_dtypes

import concourse.bass as bass
import concourse.tile as tile
from concourse import bacc, mybir
from concourse.bass_utils import run_bass_kernel_spmd

N_CORES = 8
SQ = 4096
SK = 77
SKP = 80  # SK padded for kT psum tiles
E = 1024
C = 768
H = 16
D = 64
CHUNK = 512
NCHUNK = SQ // CHUNK  # 8
ET = E // 128  # 8 e-tiles
CT = C // 128  # 6 c-tiles
F32 = mybir.dt.float32
BF16 = mybir.dt.bfloat16
BF = ml_dtypes.bfloat16

_PROGRAM = None


def _build_program():
    nc = bacc.Bacc(
        "TRN2", target_bir_lowering=False, debug=False, num_devices=N_CORES
    )
    # xT pre-tiled host-side: [chunk, partition, e-tile, col].
    xT_d = nc.dram_tensor(
        "xT", [NCHUNK, 128, ET, CHUNK], BF16, kind="ExternalInput"
    ).ap()
    yT_d = nc.dram_tensor("yT", [C, SKP], BF16, kind="ExternalInput").ap()
    wq_d = nc.dram_tensor("wq", [E, E], BF16, kind="ExternalInput").ap()
    bq_d = nc.dram_tensor("bq", [E], F32, kind="ExternalInput").ap()
    wk_d = nc.dram_tensor("wk", [C, E], BF16, kind="ExternalInput").ap()
    bk_d = nc.dram_tensor("bk", [E], F32, kind="ExternalInput").ap()
    wv_d = nc.dram_tensor("wv", [C, E], BF16, kind="ExternalInput").ap()
    bv_d = nc.dram_tensor("bv", [E], F32, kind="ExternalInput").ap()
    wo_d = nc.dram_tensor("wo", [E, E], BF16, kind="ExternalInput").ap()
    bo_d = nc.dram_tensor("bo", [E], F32, kind="ExternalInput").ap()
    out_d = nc.dram_tensor("out", [SQ, E], F32, kind="ExternalOutput").ap()

    wq_r = wq_d.rearrange("(t p) n -> p t n", p=128)
    wo_r = wo_d.rearrange("(t p) n -> p t n", p=128)
    yT_r = yT_d.rearrange("(t p) n -> p t n", p=128)
    wk_r = wk_d.rearrange("(t p) n -> p t n", p=128)
    wv_r = wv_d.rearrange("(t p) n -> p t n", p=128)

    with tile.TileContext(nc) as tc, ExitStack() as ctx, nc.allow_low_precision(
        reason="bf16 pipeline; fp32 PSUM accumulation throughout"
    ):
        consts = ctx.enter_context(tc.tile_pool(name="consts", bufs=1))
        wq_sb = consts.tile([128, ET, E], BF16)
        wo_sb = consts.tile([128, ET, E], BF16)
        kT_sb = consts.tile([128, ET, SKP], BF16)
        v_sb = consts.tile([SK, H * 64], BF16)
        ones_sb = consts.tile([SK, 64], BF16)
        bq_sb = consts.tile([128, ET], F32)
        bk_sb = consts.tile([128, ET], F32)
        bv_sb = consts.tile([SK, H * 64], F32)
        bo_sb = consts.tile([128, E], F32)

        nc.any.memset(ones_sb[:], 1.0)

        ps_q = ctx.enter_context(tc.tile_pool(name="ps_q", bufs=2, space="PSUM"))
        xT_pool = ctx.enter_context(tc.tile_pool(name="xT", bufs=2))
        qT_pool = ctx.enter_context(tc.tile_pool(name="qT", bufs=2))

        # --- DMA issue order == consumption order (the prefix is HBM-
        # bandwidth-bound, so the stream must land in the order the PE
        # consumes it): yT+wk (kT matmuls, first PE work at ~4us) ->
        # wq et-0 slices + xT0 tiles (QT group 0) -> remaining wq et-blocks
        # (QT groups 1-7) -> wv (V matmuls) -> xT1/wo (chunk-0 fillers and
        # chunk-1 finals).
        def emit_qt_group(xT_sb, qT_sb, et):
            ps = ps_q.tile([128, CHUNK], F32, tag="psq")
            for t in range(ET):
                nc.tensor.matmul(
                    ps[:],
                    wq_sb[:, t, et * 128 : (et + 1) * 128],
                    xT_sb[:, t, :],
                    start=(t == 0),
                    stop=(t == ET - 1),
                )
            nc.scalar.activation(
                qT_sb[:, et, :],
                ps[:],
                mybir.ActivationFunctionType.Identity,
                bias=bq_sb[:, et : et + 1],
            )

        # --- Phase 0 + QT(0), emission interleaved with the DMA stream ---
        with tc.tile_pool(name="ph0", bufs=1) as ph0, tc.tile_pool(
            name="ph0k", bufs=4, space="PSUM"
        ) as ph0k, tc.tile_pool(name="ph0v", bufs=2, space="PSUM") as ph0v:
            yT_sb = ph0.tile([128, CT, SKP], BF16)
            wk_sb = ph0.tile([128, CT, E], BF16)
            wv_sb = ph0.tile([128, CT, E], BF16)
            # Prefix loads: each dma_start costs ~600ns of issue time on its
            # engine, and HW queues drain in parallel -- so spread the issues
            # across the (idle) engines instead of serializing on Sync.
            nc.sync.dma_start(yT_sb[:], yT_r)
            nc.sync.dma_start(wk_sb[:], wk_r)
            nc.sync.dma_start(wv_sb[:], wv_r)
            nc.scalar.dma_start(bk_sb[:], bk_d.rearrange("(t p) -> p t", p=128))
            nc.scalar.dma_start(bq_sb[:], bq_d.rearrange("(t p) -> p t", p=128))
            nc.scalar.dma_start(bv_sb[:], bv_d.partition_broadcast(SK))
            xT0_sb = xT_pool.tile([128, ET, CHUNK], BF16, tag="xT")
            for h in range(0, ET, 4):
                nc.scalar.dma_start(wq_sb[:, h : h + 4, :], wq_r[:, h : h + 4, :])
            for h in range(0, ET, 4):
                nc.sync.dma_start(
                    xT0_sb[:, h : h + 4, :], xT_d[0][:, h : h + 4, :]
                )

            # kT in et-pairs, t-major inside: the first matmuls need only
            # yT + wk[0], so PE work starts as soon as ~0.4MB has landed.
            for p in range(ET // 2):
                pskA = ph0k.tile([128, SKP], F32, tag="psk")
                pskB = ph0k.tile([128, SKP], F32, tag="psk", name="pskB")
                for t in range(CT):
                    for psk, et in ((pskA, 2 * p), (pskB, 2 * p + 1)):
                        nc.tensor.matmul(
                            psk[:],
                            wk_sb[:, t, et * 128 : (et + 1) * 128],
                            yT_sb[:, t, :],
                            start=(t == 0),
                            stop=(t == CT - 1),
                        )
                for psk, et in ((pskA, 2 * p), (pskB, 2 * p + 1)):
                    nc.scalar.activation(
                        kT_sb[:, et, :],
                        psk[:],
                        mybir.ActivationFunctionType.Identity,
                        bias=bk_sb[:, et : et + 1],
                    )

            # V next (small PE cost, needed by the first attention tail).
            for g in range(2):
                psv = ph0v.tile([SK, CHUNK], F32, tag="psv")
                for t in range(CT):
                    nc.tensor.matmul(
                        psv[:],
                        yT_sb[:, t, 0:SK],
                        wv_sb[:, t, g * CHUNK : (g + 1) * CHUNK],
                        start=(t == 0),
                        stop=(t == CT - 1),
                    )
                nc.vector.tensor_tensor(
                    v_sb[:, g * CHUNK : (g + 1) * CHUNK],
                    psv[:],
                    bv_sb[:, g * CHUNK : (g + 1) * CHUNK],
                    mybir.AluOpType.add,
                )

            # Only QT(0) group 0 before the main loop; groups 1-7 and all of
            # QT(1) drain as chunk-0 attention fillers, so chunk 0's
            # attention pipelines INTO the QT stream instead of after it.
            qT0_sb = qT_pool.tile([128, ET, CHUNK], BF16, tag="qT")
            emit_qt_group(xT0_sb, qT0_sb, 0)

        # --- Main-loop pools ---
        oT_pool = ctx.enter_context(tc.tile_pool(name="oT", bufs=2))
        exps_pool = ctx.enter_context(tc.tile_pool(name="exps", bufs=4))
        rbr_pool = ctx.enter_context(tc.tile_pool(name="rbr", bufs=3))
        outs_pool = ctx.enter_context(tc.tile_pool(name="outs", bufs=3))
        ps_s = ctx.enter_context(tc.tile_pool(name="ps_s", bufs=2, space="PSUM"))
        ps_pav = ctx.enter_context(tc.tile_pool(name="ps_pav", bufs=2, space="PSUM"))
        ps_den = ctx.enter_context(tc.tile_pool(name="ps_den", bufs=2, space="PSUM"))

        # wo needed first by final(0) during chunk 1; xT(1) by QT(1) fillers
        # inside chunk 0 — both off QT(0)/phase0's critical DMA path.
        xT_tiles = {0: xT0_sb}
        qT_tiles = {0: qT0_sb}

        def load_xT(c):
            t_ = xT_pool.tile([128, ET, CHUNK], BF16, tag="xT", name="xTn")
            nc.sync.dma_start(t_[:], xT_d[c])
            xT_tiles[c] = t_

        load_xT(1)
        nc.sync.dma_start(bo_sb[:], bo_d.partition_broadcast(128))
        for lo, hi in ((0, 4), (4, 8)):
            nc.sync.dma_start(wo_sb[:, lo:hi, :], wo_r[:, lo:hi, :])

        def emit_scores(qT_sb, et):
            psa = ps_s.tile([SK, CHUNK], F32, tag="pss")
            psb = ps_s.tile([SK, CHUNK], F32, tag="pss")
            nc.tensor.matmul(
                psa[:], kT_sb[0:64, et, 0:SK], qT_sb[0:64, et, :],
                start=True, stop=True,
            )
            nc.tensor.matmul(
                psb[:], kT_sb[64:128, et, 0:SK], qT_sb[64:128, et, :],
                start=True, stop=True,
            )
            exa = exps_pool.tile([SK, CHUNK], BF16, tag="exps")
            exb = exps_pool.tile([SK, CHUNK], BF16, tag="exps")
            nc.scalar.activation(exa[:], psa[:], mybir.ActivationFunctionType.Exp)
            nc.scalar.activation(exb[:], psb[:], mybir.ActivationFunctionType.Exp)
            return exa, exb

        def emit_tail_att(oT_sb, exa, exb, et):
            hA, hB = 2 * et, 2 * et + 1
            pav = ps_pav.tile([128, CHUNK], F32, tag="pspav")
            nc.tensor.matmul(
                pav[0:64, :], v_sb[:, hA * 64 : (hA + 1) * 64], exa[:],
                start=True, stop=True,
            )
            nc.tensor.matmul(
                pav[64:128, :], v_sb[:, hB * 64 : (hB + 1) * 64], exb[:],
                start=True, stop=True,
            )
            den = ps_den.tile([128, CHUNK], F32, tag="psden")
            nc.tensor.matmul(
                den[0:64, :], ones_sb[:], exa[:], start=True, stop=True
            )
            nc.tensor.matmul(
                den[64:128, :], ones_sb[:], exb[:], start=True, stop=True
            )
            rbr = rbr_pool.tile([128, CHUNK], F32, tag="rbr")
            nc.vector.reciprocal_approx_fast(rbr[:], den[:])
            # Normalization fused into the PSUM eviction.
            nc.vector.tensor_tensor(
                oT_sb[:, et, :], pav[:], rbr[:], mybir.AluOpType.mult
            )

        def emit_final_group(c, oT_sb, i):
            qt, n0 = i // 2, (i % 2) * CHUNK
            ps = ps_q.tile([128, CHUNK], F32, tag="psq", name="psf")
            for t in range(ET):
                nc.tensor.matmul(
                    ps[:],
                    oT_sb[:, t, qt * 128 : (qt + 1) * 128],
                    wo_sb[:, t, n0 : n0 + CHUNK],
                    start=(t == 0),
                    stop=(t == ET - 1),
                )
            o_sb = outs_pool.tile([128, CHUNK], F32, tag="osb")
            nc.vector.tensor_tensor(
                o_sb[:], ps[:], bo_sb[:, n0 : n0 + CHUNK], mybir.AluOpType.add
            )
            r0 = c * CHUNK + qt * 128
            nc.sync.dma_start(out_d[r0 : r0 + 128, n0 : n0 + CHUNK], o_sb[:])

        # Chunk-0 fillers: QT(0) groups 1-7 then QT(1) groups 0-7, two per
        # attention iteration. QT(0) group et is always emitted >= 1
        # iteration before scores(0, et) consumes it.
        qT_tiles[1] = qT_pool.tile([128, ET, CHUNK], BF16, tag="qT", name="qTn")
        fillers = [
            (lambda et=et: emit_qt_group(xT_tiles[0], qT_tiles[0], et))
            for et in range(1, ET)
        ] + [
            (lambda et=et: emit_qt_group(xT_tiles[1], qT_tiles[1], et))
            for et in range(ET)
        ]
        for c in range(NCHUNK):
            if 1 <= c < NCHUNK - 1:
                load_xT(c + 1)
            qT_sb = qT_tiles[c]
            oT_sb = oT_pool.tile([128, ET, CHUNK], BF16, tag="oT")
            exs = [None] * ET
            for et in range(ET):
                exs[et] = emit_scores(qT_sb, et)
                if et >= 1:
                    emit_tail_att(oT_sb, *exs[et - 1], et - 1)
                if c == 0:
                    for _ in range(2):
                        if fillers:
                            fillers.pop(0)()
                else:
                    emit_final_group(c - 1, prev_oT, et)
            emit_tail_att(oT_sb, *exs[ET - 1], ET - 1)
            prev_oT = oT_sb
            # QT for chunk c+1 (c=0's was emitted inline above).
            if 1 <= c < NCHUNK - 1:
                qT_tiles[c + 1] = qT_pool.tile([128, ET, CHUNK], BF16, tag="qT", name="qTn")
                for et in range(ET):
                    emit_qt_group(xT_tiles[c + 1], qT_tiles[c + 1], et)
        # Tail: last chunk's output projection.
        for i in range(ET):
            emit_final_group(NCHUNK - 1, prev_oT, i)

    nc.compile()
    return nc


def _get_program():
    global _PROGRAM
    if _PROGRAM is None:
        _PROGRAM = _build_program()
    return _PROGRAM


def kernel(x, y, wq, bq, wk, bk, wv, bv, wo, bo):
    x = np.asarray(x, dtype=np.float32)
    y = np.asarray(y, dtype=np.float32)
    wq = np.asarray(wq, dtype=np.float32)
    bq = np.asarray(bq, dtype=np.float32)
    wk = np.asarray(wk, dtype=np.float32)
    bk = np.asarray(bk, dtype=np.float32)
    wv = np.asarray(wv, dtype=np.float32)
    bv = np.asarray(bv, dtype=np.float32)
    wo = np.asarray(wo, dtype=np.float32)
    bo = np.asarray(bo, dtype=np.float32)

    scale = np.float32(1.0 / np.sqrt(D))
    wq_s = (wq * scale).astype(BF)
    bq_s = (bq * scale).astype(np.float32)
    wk_b = wk.astype(BF)
    wv_b = wv.astype(BF)
    wo_b = wo.astype(BF)

    nc = _get_program()
    in_maps = []
    for b in range(N_CORES):
        # [E, Sq] -> [chunk, partition, e-tile, col], contiguous per chunk.
        xT = np.ascontiguousarray(
            x[b].T.reshape(ET, 128, NCHUNK, CHUNK).transpose(2, 1, 0, 3)
        ).astype(BF)
        yT = np.zeros((C, SKP), dtype=np.float32)
        yT[:, :SK] = y[b].T
        yT = yT.astype(BF)
        in_maps.append(
            {
                "xT": xT,
                "yT": yT,
                "wq": wq_s,
                "bq": bq_s,
                "wk": wk_b,
                "bk": bk.astype(np.float32),
                "wv": wv_b,
                "bv": bv.astype(np.float32),
                "wo": wo_b,
                "bo": bo,
            }
        )

    trace = bool(int(os.environ.get("KERNEL_TRACE", "0")))
    kwargs = {}
    if trace:
        kwargs = {"trace": True, "tmpdir": os.environ.get("KERNEL_TRACE_DIR")}
    try:
        res = run_bass_kernel_spmd(nc, in_maps, list(range(N_CORES)), **kwargs)
    except Exception:
        # The axon-tunneled devices occasionally report a transient
        # NRT_EXEC_UNIT_UNRECOVERABLE; a retry on the same executable has
        # been observed to succeed.
        res = run_bass_kernel_spmd(nc, in_maps, list(range(N_CORES)), **kwargs)
    if trace:
        kernel.last_exec_time_ns = res.exec_time_ns
        kernel.last_results = res
    out = np.stack([res.results[b]["out"] for b in range(N_CORES)])
    return np.ascontiguousarray(out)



# revision 2
# speedup vs baseline: 1.1484x; 1.1484x over previous
"""Cross-attention kernel for Trainium2 (8 NeuronCores, data-parallel over batch).

Problem (hardcoded): B=8, Sq=4096, Sk=77, E=1024, C=768, H=16 heads, D=64.

    q = x @ wq + bq; k = y @ wk + bk; v = y @ wv + bv
    out = softmax(q k^T / sqrt(D)) v @ wo + bo

Sharding: batch element b -> core b. No collectives.

All matmul operands are bf16 (1 cycle/row PE rate); PSUM accumulation fp32.

Key performance facts (measured on HW):
  - Any custom-ucode DVE op (e.g. reciprocal_approx_fast) in the NEFF drops
    the PE clock from 2.4 to 2.0 GHz for the WHOLE run (454 vs 379 ns dur on
    N=512 matmuls; 257 vs 216 ns issue spacing). The softmax reciprocal is
    therefore computed with STANDARD DVE ops: seed = ~bits(den) (bitwise-xor
    -1 of the fp32 pattern) then one Newton step, all fused into 3
    tensor_scalar/scalar_tensor_tensor ops + the eviction multiply
    (max rel err ~1.7e-3, measured).
  - K<128 matmul pairs on distinct PE row/col groups (tile_position) run
    concurrently; scores (K=64, rows 0/64) and pav/den (M=64, cols 0/64)
    pairs each retire in one N=512 slot.
  - Per-MM issue floor is N/2.4GHz + ~3ns; LDWEIGHTS is fully hidden.

Per-core pipeline (all matmuls contract over the SBUF partition dim):
  - Activations feature-major: xT[E, Sq] prepared host-side; 1/sqrt(D)
    folded into wq/bq host-side.
  - Startup: xT0/wq arrive as fine-grained DMA slices so the first QT
    matmul issues at ~2us (was ~17us): QT(0) groups 0-1 first, then
    phase 0 (kT via wk tiles, V via yT tiles), then the main loop; chunk-0
    fillers = QT(0) groups 2-7 + QT(1) groups 0-7 keep the PE busy during
    the scores->exp->attnV ScalarE round-trips.
  - scores^T[Sk, q] per head pair: two K=64 matmuls on PE row groups.
  - exp on ScalarE (no max-subtraction: scores are O(6), fp32 PSUM in,
    bf16 out).
  - attn@V + denominators col-paired into PSUM banks as in the baseline.
  - Normalization: notx = den ^ -1 (bitwise), m = den*S0*notx,
    r = (m-S1)*notx, then oT = pav * (-S0) * r fused into the PSUM
    eviction (DVE scalar_tensor_tensor).
  - out[q, E] = matmul(lhsT=oT tiles, rhs=wo tiles), bo added during
    eviction (DVE), fp32 out.
"""

import os
from contextlib import ExitStack

import numpy as np
import ml_dtypes

import concourse.bass as bass
import concourse.tile as tile
from concourse import bacc, mybir
from concourse.bass_utils import run_bass_kernel_spmd

N_CORES = 8
SQ = 4096
SK = 77
SKP = 80  # SK padded for kT psum tiles
E = 1024
C = 768
H = 16
D = 64
CHUNK = 512
NCHUNK = SQ // CHUNK  # 8
ET = E // 128  # 8 e-tiles
CT = C // 128  # 6 c-tiles
F32 = mybir.dt.float32
I32 = mybir.dt.int32
BF16 = mybir.dt.bfloat16
BF = ml_dtypes.bfloat16

# Newton-reciprocal constants (same as concourse reciprocal_approx_fast seed)
RS0 = -0.23549792
RS1 = 2.0017324

_PROGRAM = None


def _build_program():
    nc = bacc.Bacc(
        "TRN2", target_bir_lowering=False, debug=False, num_devices=N_CORES
    )
    # xT pre-tiled host-side: [chunk, partition, e-tile, col].
    xT_d = nc.dram_tensor(
        "xT", [NCHUNK, 128, ET, CHUNK], BF16, kind="ExternalInput"
    ).ap()
    yT_d = nc.dram_tensor("yT", [C, SKP], BF16, kind="ExternalInput").ap()
    wq_d = nc.dram_tensor("wq", [E, E], BF16, kind="ExternalInput").ap()
    bq_d = nc.dram_tensor("bq", [E], F32, kind="ExternalInput").ap()
    wk_d = nc.dram_tensor("wk", [C, E], BF16, kind="ExternalInput").ap()
    bk_d = nc.dram_tensor("bk", [E], F32, kind="ExternalInput").ap()
    wv_d = nc.dram_tensor("wv", [C, E], BF16, kind="ExternalInput").ap()
    bv_d = nc.dram_tensor("bv", [E], F32, kind="ExternalInput").ap()
    wo_d = nc.dram_tensor("wo", [E, E], BF16, kind="ExternalInput").ap()
    bo_d = nc.dram_tensor("bo", [E], F32, kind="ExternalInput").ap()
    out_d = nc.dram_tensor("out", [SQ, E], F32, kind="ExternalOutput").ap()

    wq_r = wq_d.rearrange("(t p) n -> p t n", p=128)
    wo_r = wo_d.rearrange("(t p) n -> p t n", p=128)
    yT_r = yT_d.rearrange("(t p) n -> p t n", p=128)
    wk_r = wk_d.rearrange("(t p) n -> p t n", p=128)
    wv_r = wv_d.rearrange("(t p) n -> p t n", p=128)

    with tile.TileContext(nc) as tc, ExitStack() as ctx, nc.allow_low_precision(
        reason="bf16 pipeline; fp32 PSUM accumulation throughout"
    ):
        consts = ctx.enter_context(tc.tile_pool(name="consts", bufs=1))
        wq_sb = consts.tile([128, ET, E], BF16)
        wo_sb = consts.tile([128, ET, E], BF16)
        kT_sb = consts.tile([128, ET, SKP], BF16)
        v_sb = consts.tile([SK, H * 64], BF16)
        ones_sb = consts.tile([SK, 64], BF16)
        bq_sb = consts.tile([128, ET], F32)
        bk_sb = consts.tile([128, ET], F32)
        bv_sb = consts.tile([SK, H * 64], F32)
        bo_sb = consts.tile([128, E], F32)

        nc.any.memset(ones_sb[:], 1.0)

        ps_q = ctx.enter_context(tc.tile_pool(name="ps_q", bufs=2, space="PSUM"))
        xT_pool = ctx.enter_context(tc.tile_pool(name="xT", bufs=2))
        qT_pool = ctx.enter_context(tc.tile_pool(name="qT", bufs=2))

        def emit_qt_group(xT_sb, qT_sb, et):
            ps = ps_q.tile([128, CHUNK], F32, tag="psq")
            for t in range(ET):
                nc.tensor.matmul(
                    ps[:],
                    wq_sb[:, t, et * 128 : (et + 1) * 128],
                    xT_sb[:, t, :],
                    start=(t == 0),
                    stop=(t == ET - 1),
                )
            nc.scalar.activation(
                qT_sb[:, et, :],
                ps[:],
                mybir.ActivationFunctionType.Identity,
                bias=bq_sb[:, et : et + 1],
            )

        # --- Startup DMA stream -------------------------------------------
        # The first PE work is QT(0) group 0, which needs wq n-slice 0 and
        # xT0 t-slices in t order. Issue those first, fine-grained, so the
        # first matmul launches at ~2us. phase0 weights (yT/wk/wv, ~3.2MB)
        # stream behind on the sync queue and land before the kT matmuls
        # (emitted after QT(0) groups 0-1) need them.
        xT0_sb = xT_pool.tile([128, ET, CHUNK], BF16, tag="xT")
        nc.scalar.dma_start(bq_sb[:], bq_d.rearrange("(t p) -> p t", p=128))
        for t in range(ET):
            nc.sync.dma_start(xT0_sb[:, t : t + 1, :], xT_d[0][:, t : t + 1, :])
        for et in range(ET):
            nc.scalar.dma_start(
                wq_sb[:, :, et * 128 : (et + 1) * 128],
                wq_r[:, :, et * 128 : (et + 1) * 128],
            )

        qT0_sb = qT_pool.tile([128, ET, CHUNK], BF16, tag="qT")
        emit_qt_group(xT0_sb, qT0_sb, 0)
        emit_qt_group(xT0_sb, qT0_sb, 1)

        # phase0 weight DMAs + remaining constants.
        with tc.tile_pool(name="ph0", bufs=1) as ph0, tc.tile_pool(
            name="ph0k", bufs=4, space="PSUM"
        ) as ph0k, tc.tile_pool(name="ph0v", bufs=2, space="PSUM") as ph0v:
            yT_sb = ph0.tile([128, CT, SKP], BF16)
            wk_sb = ph0.tile([128, CT, E], BF16)
            wv_sb = ph0.tile([128, CT, E], BF16)
            nc.sync.dma_start(yT_sb[:], yT_r)
            nc.sync.dma_start(wk_sb[:], wk_r)
            nc.sync.dma_start(wv_sb[:], wv_r)
            nc.scalar.dma_start(bk_sb[:], bk_d.rearrange("(t p) -> p t", p=128))
            nc.scalar.dma_start(bv_sb[:], bv_d.partition_broadcast(SK))

            # kT in et-pairs, t-major inside.
            for p in range(ET // 2):
                pskA = ph0k.tile([128, SKP], F32, tag="psk")
                pskB = ph0k.tile([128, SKP], F32, tag="psk", name="pskB")
                for t in range(CT):
                    for psk, et in ((pskA, 2 * p), (pskB, 2 * p + 1)):
                        nc.tensor.matmul(
                            psk[:],
                            wk_sb[:, t, et * 128 : (et + 1) * 128],
                            yT_sb[:, t, :],
                            start=(t == 0),
                            stop=(t == CT - 1),
                        )
                for psk, et in ((pskA, 2 * p), (pskB, 2 * p + 1)):
                    nc.scalar.activation(
                        kT_sb[:, et, :],
                        psk[:],
                        mybir.ActivationFunctionType.Identity,
                        bias=bk_sb[:, et : et + 1],
                    )

            # V next (small PE cost, needed by the first attention tail).
            for g in range(2):
                psv = ph0v.tile([SK, CHUNK], F32, tag="psv")
                for t in range(CT):
                    nc.tensor.matmul(
                        psv[:],
                        yT_sb[:, t, 0:SK],
                        wv_sb[:, t, g * CHUNK : (g + 1) * CHUNK],
                        start=(t == 0),
                        stop=(t == CT - 1),
                    )
                nc.vector.tensor_tensor(
                    v_sb[:, g * CHUNK : (g + 1) * CHUNK],
                    psv[:],
                    bv_sb[:, g * CHUNK : (g + 1) * CHUNK],
                    mybir.AluOpType.add,
                )

        # --- Main-loop pools ---
        oT_pool = ctx.enter_context(tc.tile_pool(name="oT", bufs=2))
        exps_pool = ctx.enter_context(tc.tile_pool(name="exps", bufs=4))
        nr_pool = ctx.enter_context(tc.tile_pool(name="nr", bufs=6))
        outs_pool = ctx.enter_context(tc.tile_pool(name="outs", bufs=3))
        ps_s = ctx.enter_context(tc.tile_pool(name="ps_s", bufs=2, space="PSUM"))
        ps_pav = ctx.enter_context(tc.tile_pool(name="ps_pav", bufs=2, space="PSUM"))
        ps_den = ctx.enter_context(tc.tile_pool(name="ps_den", bufs=2, space="PSUM"))

        # wo needed first by final(0) during chunk 1; xT(1) by QT(1) fillers
        # inside chunk 0.
        xT_tiles = {0: xT0_sb}
        qT_tiles = {0: qT0_sb}

        def load_xT(c):
            t_ = xT_pool.tile([128, ET, CHUNK], BF16, tag="xT", name="xTn")
            nc.sync.dma_start(t_[:], xT_d[c])
            xT_tiles[c] = t_

        load_xT(1)
        nc.sync.dma_start(bo_sb[:], bo_d.partition_broadcast(128))
        for lo, hi in ((0, 4), (4, 8)):
            nc.scalar.dma_start(wo_sb[:, lo:hi, :], wo_r[:, lo:hi, :])

        def emit_scores(qT_sb, et):
            psa = ps_s.tile([SK, CHUNK], F32, tag="pss")
            psb = ps_s.tile([SK, CHUNK], F32, tag="pss")
            nc.tensor.matmul(
                psa[:], kT_sb[0:64, et, 0:SK], qT_sb[0:64, et, :],
                start=True, stop=True,
            )
            nc.tensor.matmul(
                psb[:], kT_sb[64:128, et, 0:SK], qT_sb[64:128, et, :],
                start=True, stop=True,
            )
            exa = exps_pool.tile([SK, CHUNK], BF16, tag="exps")
            exb = exps_pool.tile([SK, CHUNK], BF16, tag="exps")
            nc.scalar.activation(exa[:], psa[:], mybir.ActivationFunctionType.Exp)
            nc.scalar.activation(exb[:], psb[:], mybir.ActivationFunctionType.Exp)
            return exa, exb

        def emit_tail_att(oT_sb, exa, exb, et):
            hA, hB = 2 * et, 2 * et + 1
            pav = ps_pav.tile([128, CHUNK], F32, tag="pspav")
            nc.tensor.matmul(
                pav[0:64, :], v_sb[:, hA * 64 : (hA + 1) * 64], exa[:],
                start=True, stop=True,
            )
            nc.tensor.matmul(
                pav[64:128, :], v_sb[:, hB * 64 : (hB + 1) * 64], exb[:],
                start=True, stop=True,
            )
            den = ps_den.tile([128, CHUNK], F32, tag="psden")
            nc.tensor.matmul(
                den[0:64, :], ones_sb[:], exa[:], start=True, stop=True
            )
            nc.tensor.matmul(
                den[64:128, :], ones_sb[:], exb[:], start=True, stop=True
            )
            # Newton reciprocal with standard DVE ops (no custom ucode):
            #   notx = ~bits(den); m = den*RS0*notx; r = (m-RS1)*notx
            #   1/den ~= -RS0 * r   (folded into the eviction multiply)
            notx = nr_pool.tile([128, CHUNK], F32, tag="notx", name="notx")
            nc.vector.tensor_scalar(
                notx[:].bitcast(I32), den[:].bitcast(I32), -1, None,
                mybir.AluOpType.bitwise_xor,
            )
            m = nr_pool.tile([128, CHUNK], F32, tag="m", name="m")
            nc.vector.scalar_tensor_tensor(
                m[:], den[:], RS0, notx[:],
                mybir.AluOpType.mult, mybir.AluOpType.mult,
            )
            r = nr_pool.tile([128, CHUNK], F32, tag="r", name="r")
            nc.vector.scalar_tensor_tensor(
                r[:], m[:], RS1, notx[:],
                mybir.AluOpType.subtract, mybir.AluOpType.mult,
            )
            # Normalization fused into the PSUM eviction: oT = pav*(-RS0)*r.
            nc.vector.scalar_tensor_tensor(
                oT_sb[:, et, :], pav[:], -RS0, r[:],
                mybir.AluOpType.mult, mybir.AluOpType.mult,
            )

        def emit_final_group(c, oT_sb, i):
            qt, n0 = i // 2, (i % 2) * CHUNK
            ps = ps_q.tile([128, CHUNK], F32, tag="psq", name="psf")
            for t in range(ET):
                nc.tensor.matmul(
                    ps[:],
                    oT_sb[:, t, qt * 128 : (qt + 1) * 128],
                    wo_sb[:, t, n0 : n0 + CHUNK],
                    start=(t == 0),
                    stop=(t == ET - 1),
                )
            o_sb = outs_pool.tile([128, CHUNK], F32, tag="osb")
            nc.vector.tensor_tensor(
                o_sb[:], ps[:], bo_sb[:, n0 : n0 + CHUNK], mybir.AluOpType.add
            )
            r0 = c * CHUNK + qt * 128
            nc.sync.dma_start(out_d[r0 : r0 + 128, n0 : n0 + CHUNK], o_sb[:])

        # Chunk-0 fillers: QT(0) groups 2-7 then QT(1) groups 0-7, two per
        # attention iteration.
        qT_tiles[1] = qT_pool.tile([128, ET, CHUNK], BF16, tag="qT", name="qTn")
        fillers = [
            (lambda et=et: emit_qt_group(xT_tiles[0], qT_tiles[0], et))
            for et in range(2, ET)
        ] + [
            (lambda et=et: emit_qt_group(xT_tiles[1], qT_tiles[1], et))
            for et in range(ET)
        ]
        for c in range(NCHUNK):
            if 1 <= c < NCHUNK - 1:
                load_xT(c + 1)
            qT_sb = qT_tiles[c]
            oT_sb = oT_pool.tile([128, ET, CHUNK], BF16, tag="oT")
            exs = [None] * ET
            for et in range(ET):
                exs[et] = emit_scores(qT_sb, et)
                if et >= 1:
                    emit_tail_att(oT_sb, *exs[et - 1], et - 1)
                if c == 0:
                    for _ in range(2):
                        if fillers:
                            fillers.pop(0)()
                else:
                    emit_final_group(c - 1, prev_oT, et)
            emit_tail_att(oT_sb, *exs[ET - 1], ET - 1)
            prev_oT = oT_sb
            # QT for chunk c+1 (c=0's was emitted inline above).
            if 1 <= c < NCHUNK - 1:
                qT_tiles[c + 1] = qT_pool.tile([128, ET, CHUNK], BF16, tag="qT", name="qTn")
                for et in range(ET):
                    emit_qt_group(xT_tiles[c + 1], qT_tiles[c + 1], et)
        # Tail: last chunk's output projection.
        for i in range(ET):
            emit_final_group(NCHUNK - 1, prev_oT, i)

    nc.compile()
    return nc


def _get_program():
    global _PROGRAM
    if _PROGRAM is None:
        _PROGRAM = _build_program()
    return _PROGRAM


def kernel(x, y, wq, bq, wk, bk, wv, bv, wo, bo):
    x = np.asarray(x, dtype=np.float32)
    y = np.asarray(y, dtype=np.float32)
    wq = np.asarray(wq, dtype=np.float32)
    bq = np.asarray(bq, dtype=np.float32)
    wk = np.asarray(wk, dtype=np.float32)
    bk = np.asarray(bk, dtype=np.float32)
    wv = np.asarray(wv, dtype=np.float32)
    bv = np.asarray(bv, dtype=np.float32)
    wo = np.asarray(wo, dtype=np.float32)
    bo = np.asarray(bo, dtype=np.float32)

    scale = np.float32(1.0 / np.sqrt(D))
    wq_s = (wq * scale).astype(BF)
    bq_s = (bq * scale).astype(np.float32)
    wk_b = wk.astype(BF)
    wv_b = wv.astype(BF)
    wo_b = wo.astype(BF)

    nc = _get_program()
    in_maps = []
    for b in range(N_CORES):
        # [E, Sq] -> [chunk, partition, e-tile, col], contiguous per chunk.
        xT = np.ascontiguousarray(
            x[b].T.reshape(ET, 128, NCHUNK, CHUNK).transpose(2, 1, 0, 3)
        ).astype(BF)
        yT = np.zeros((C, SKP), dtype=np.float32)
        yT[:, :SK] = y[b].T
        yT = yT.astype(BF)
        in_maps.append(
            {
                "xT": xT,
                "yT": yT,
                "wq": wq_s,
                "bq": bq_s,
                "wk": wk_b,
                "bk": bk.astype(np.float32),
                "wv": wv_b,
                "bv": bv.astype(np.float32),
                "wo": wo_b,
                "bo": bo,
            }
        )

    trace = bool(int(os.environ.get("KERNEL_TRACE", "0")))
    kwargs = {}
    if trace:
        kwargs = {"trace": True, "tmpdir": os.environ.get("KERNEL_TRACE_DIR")}
    try:
        res = run_bass_kernel_spmd(nc, in_maps, list(range(N_CORES)), **kwargs)
    except Exception:
        # The axon-tunneled devices occasionally report a transient
        # NRT_EXEC_UNIT_UNRECOVERABLE; a retry on the same executable has
        # been observed to succeed.
        res = run_bass_kernel_spmd(nc, in_maps, list(range(N_CORES)), **kwargs)
    if trace:
        kernel.last_exec_time_ns = res.exec_time_ns
        kernel.last_results = res
    out = np.stack([res.results[b]["out"] for b in range(N_CORES)])
    return np.ascontiguousarray(out)
